# revision 10
# baseline (speedup 1.0000x reference)
"""Trainium2 Bass kernel for nn_CSB (dense_transformer).

Reference computation (per sample b of B=4, N=16384, C=384, d=192, H=W=128,
M=N/16=1024):
  x1 = x[..., :d]; x2 = x[..., d:]
  x2c  = conv4x4s4(x2 as [d,H,W]) + conv_b            # [d, M]
  gate = sigmoid(x1 @ x2c)                            # [N, M]
  sp   = gate @ x2c.T                                 # [N, d]
  att  = softmax(x1.T @ x2, axis over first d)        # [d, d]
  ch   = x2 @ att                                     # [N, d]
  cat  = [sp, ch]; ln = LN(cat) * ln_w + ln_b
  out  = (ln @ proj_w.T + proj_b).T                   # [C, N]

Sharding: 8 cores = 4 samples x 2 N-halves. Each core takes the FULL sample
x[b] (s-matrix and conv need all N; computed redundantly in each half-pair)
and produces out[b][:, half*8192:(half+1)*8192].

All GEMMs run in float32r (fp32 storage, ~13-bit-mantissa multiply at full
PE rate for moving free-dim >= 256); PSUM accumulation is fp32.

Layout strategy (everything transposed, n on the free axis):
  - x2T (channels-major x2) built on-chip via PE transposes; feeds the conv
    (as strided patch views), chT, and (x1T per block) the gate.
  - gateT[m,n] = sigmoid(x2c.T @ x1T); spT[d,n] = x2cT.T @ gateT;
    chT[e,n] = att.T(d-major) @ x2T  ->  catT in [c,n] layout.
  - LN over partitions via ones-vector matmuls (sum / sum-of-squares),
    folded into the projection:
      out = (pwTs.T @ catT - pwsum x mu) * rstd_bc + bias2
    with pwTs = proj_w.T scaled by ln_w, bias2 = proj_w @ ln_b + proj_b.
"""

import sys
import types

_m = types.ModuleType("antenv.axon_hooks")
_m.get_axon_ntff_profile_hook = lambda: None
sys.modules.setdefault("antenv.axon_hooks", _m)

import numpy as np

import concourse.bacc as bacc
import concourse.mybir as mybir
import concourse.tile as tile
from concourse.masks import make_identity

F32 = mybir.dt.float32
F32R = mybir.dt.float32r
AF = mybir.ActivationFunctionType
OP = mybir.AluOpType

B = 4
N = 16384
C = 384
D = 192  # C // 2
M = 1024  # N // 16
NH = 8192  # N // 2, rows per core
NBLK = 512  # n-columns per main-loop block
NBLOCKS = NH // NBLK  # 16
EPS = 1e-5


def build_nc():
    """Build the per-core program. Every core "owns" rows 0:8192 of its
    x_full; cores handling the second N-half receive x_full with the two
    halves swapped (the s-matrix/attention are row-permutation invariant,
    and the conv's induced m-permutation cancels inside sp = sum_m gate*x2c,
    so the computed rows are exactly the owned rows)."""
    nc = bacc.Bacc(None, target_bir_lowering=False)

    x_full = nc.dram_tensor("x_full", [N, C], F32R, kind="ExternalInput")
    convw = nc.dram_tensor("convw", [D, 16, D], F32R, kind="ExternalInput")
    convb = nc.dram_tensor("convb", [D], F32, kind="ExternalInput")
    lnw_d = nc.dram_tensor("lnw", [C], F32, kind="ExternalInput")
    lnb_d = nc.dram_tensor("lnb", [C], F32R, kind="ExternalInput")
    pwT_d = nc.dram_tensor("pwT", [C, C], F32R, kind="ExternalInput")
    pb_d = nc.dram_tensor("pb", [C], F32, kind="ExternalInput")
    out_part = nc.dram_tensor("out_part", [C, NH], F32, kind="ExternalOutput")


    with tile.TileContext(nc) as tc:
        import contextlib

        with contextlib.ExitStack() as top:
            const = top.enter_context(tc.tile_pool(name="const", bufs=1))
            big = top.enter_context(tc.tile_pool(name="big", bufs=1))

            # ---------------- constants ----------------
            ident_f = const.tile([128, 128], F32, tag="ident_f")
            make_identity(nc, ident_f[:])
            identr = const.tile([128, 128], F32R, tag="identr")
            nc.vector.tensor_copy(identr[:], ident_f[:])

            ones_f = const.tile([128, 1], F32, tag="ones_f")
            nc.gpsimd.memset(ones_f[:], 1.0)
            ones_col = const.tile([128, 1], F32R, tag="ones_col")
            nc.vector.tensor_copy(ones_col[:], ones_f[:])
            eps_sb = const.tile([1, 1], F32, tag="eps_sb")
            nc.gpsimd.memset(eps_sb[:], EPS)

            # per-channel vectors as [128, k] column stacks
            lnw_sb = const.tile([128, 3], F32, tag="lnw_sb")
            nc.sync.dma_start(lnw_sb[:], lnw_d.ap().rearrange("(o p) -> p o", p=128))
            lnb_sb = const.tile([128, 3], F32R, tag="lnb_sb")
            nc.sync.dma_start(lnb_sb[:], lnb_d.ap().rearrange("(o p) -> p o", p=128))
            pb_sb = const.tile([128, 3], F32, tag="pb_sb")
            nc.sync.dma_start(pb_sb[:], pb_d.ap().rearrange("(o p) -> p o", p=128))
            convb_sb = const.tile([128, 2], F32, tag="convb_sb")
            nc.sync.dma_start(convb_sb[:, 0:1], convb.ap()[0:128, None])
            nc.sync.dma_start(convb_sb[0:64, 1:2], convb.ap()[128:192, None])

            # proj weights: pwT [c, o]; pwTs = pwT * ln_w[c]; bias2 = P@lnb + pb
            pwTs = [
                const.tile([128, C], F32R, tag=f"pwTs{i}", name=f"pwTs{i}")
                for i in range(3)
            ]
            with tc.tile_pool(name="pwload", bufs=1) as pwload, \
                 tc.tile_pool(name="pwpsum", bufs=1, space="PSUM") as pwpsum:
                pwt_raw = [
                    pwload.tile([128, C], F32R, tag=f"pwt{i}", name=f"pwt{i}")
                    for i in range(3)
                ]
                for i in range(3):
                    nc.sync.dma_start(
                        pwt_raw[i][:], pwT_d.ap()[128 * i : 128 * (i + 1), :]
                    )
                # bias2 = proj_w @ ln_b + proj_b  (per-o, [128, 3])
                bias2_sb = const.tile([128, 3], F32, tag="bias2_sb")
                for oc in range(3):
                    psb = pwpsum.tile([128, 1], F32, tag="psb", name="psb")
                    for i in range(3):
                        # tiny free dims violate fp32r ISA restrictions; run
                        # these one-time matmuls as plain fp32 (bitcast)
                        nc.tensor.matmul(
                            psb[:],
                            pwt_raw[i][:, 128 * oc : 128 * (oc + 1)].bitcast(F32),
                            lnb_sb[:, i : i + 1].bitcast(F32),
                            start=(i == 0),
                            stop=(i == 2),
                        )
                    nc.scalar.activation(
                        bias2_sb[:, oc : oc + 1], psb[:], AF.Identity,
                        bias=pb_sb[:, oc : oc + 1],
                    )
                # pwTs = pwt * lnw (per-partition scalar on c)
                for i in range(3):
                    nc.vector.tensor_scalar_mul(
                        pwTs[i][:], pwt_raw[i][:], lnw_sb[:, i : i + 1]
                    )
                # pwsumneg_row[1, C] = -sum_c pwTs[c, o]
                pwsumneg_row = const.tile([1, C], F32R, tag="pwsumneg_row")
                pssum = pwpsum.tile([1, C], F32, tag="pssum", name="pssum")
                for i in range(3):
                    nc.tensor.matmul(
                        pssum[:], ones_f[:], pwTs[i][:].bitcast(F32),
                        start=(i == 0), stop=(i == 2),
                    )
                nc.vector.tensor_scalar_mul(pwsumneg_row[:], pssum[:], -1.0)

            # ---------------- resident big tensors ----------------
            # x2T0: [128, N]   channels 0:128 of x2, all N columns
            # x2T1p: [128, NH] channels 128:192 of x2; n-half j lives on
            #        partitions 64j:64j+64
            x2T0 = big.tile([128, N], F32R, tag="x2T0")
            x2T1p = big.tile([128, NH], F32R, tag="x2T1p")
            x2c0 = big.tile([128, M], F32R, tag="x2c0")  # conv out, o 0:128
            x2c1 = big.tile([64, M], F32R, tag="x2c1")  # conv out, o 128:192
            x2cT = big.tile([128, 8, D], F32R, tag="x2cT")  # [m-chunk, mc, o]
            att0 = big.tile([128, D], F32R, tag="att0")  # att[d0:128, e]
            att1 = big.tile([64, D], F32R, tag="att1")  # att[d 128:192, e]
            attT0 = big.tile([128, D], F32R, tag="attT0")  # attT[e0:128, d]
            attT1 = big.tile([64, D], F32R, tag="attT1")

            # ---------------- phase 0: stream x, s-matmuls, x2 transposes ----
            with tc.tile_pool(name="p0xt", bufs=3) as p0xt, \
                 tc.tile_pool(name="p0tp", bufs=2, space="PSUM") as p0tp, \
                 tc.tile_pool(name="p0s", bufs=1, space="PSUM") as p0s:
                sT0 = p0s.tile([128, 256], F32, tag="sT0", name="sT0")
                sT1 = p0s.tile([64, 256], F32, tag="sT1", name="sT1")
                for q in range(32):  # quads of 4 n-tiles
                    xt = p0xt.tile([128, 4, C], F32R, tag="xt", name="xt")
                    nc.sync.dma_start(
                        xt[:],
                        x_full.ap()[512 * q : 512 * (q + 1), :].rearrange(
                            "(t p) c -> p t c", p=128
                        ),
                    )
                    for j in range(4):
                        t = 4 * q + j
                        xtj = xt[:, j, :]
                        # s-matrix partials: sT[e, d] += x2_tile.T @ x1_tile
                        nc.tensor.matmul(
                            sT0[:], xtj[:, 192:320], xtj[:, 0:256],
                            start=(t == 0), stop=(t == N // 128 - 1),
                        )
                        nc.tensor.matmul(
                            sT1[:], xtj[:, 320:384], xtj[:, 0:256],
                            start=(t == 0), stop=(t == N // 128 - 1),
                        )
                        # x2 transposes into resident x2T
                        tpa = p0tp.tile([128, 128], F32R, tag="tpa", name="tpa")
                        nc.tensor.transpose(tpa[:], xtj[:, 192:320], identr[:])
                        nc.vector.tensor_copy(
                            x2T0[:, 128 * t : 128 * (t + 1)], tpa[:]
                        )
                        tpb = p0tp.tile([64, 128], F32R, tag="tpb", name="tpb")
                        nc.tensor.transpose(tpb[:], xtj[:, 320:384], identr[:])
                        nhalf_t = t // 64  # which n-half this tile is in
                        nc.vector.tensor_copy(
                            x2T1p[
                                64 * nhalf_t : 64 * nhalf_t + 64,
                                128 * (t % 64) : 128 * (t % 64) + 128,
                            ],
                            tpb[:],
                        )

                # ---------------- softmax over d (free axis of sT) ----------
                with tc.tile_pool(name="smx", bufs=1) as smx:
                    for (sps, ep, attT) in ((sT0, 128, attT0), (sT1, 64, attT1)):
                        mxn = smx.tile([ep, 1], F32, tag="mxn", name="mxn")
                        nc.vector.tensor_reduce(
                            mxn[:], sps[:ep, 0:D], mybir.AxisListType.X,
                            OP.max, negate=True,
                        )
                        expv = smx.tile([ep, D], F32R, tag="expv", name="expv")
                        nc.scalar.activation(
                            expv[:], sps[:ep, 0:D], AF.Exp, bias=mxn[:],
                        )
                        z = smx.tile([ep, 1], F32, tag="z", name="z")
                        nc.vector.reduce_sum(z[:], expv[:], axis=mybir.AxisListType.X)
                        rz = smx.tile([ep, 1], F32, tag="rz", name="rz")
                        nc.vector.reciprocal(rz[:], z[:])
                        nc.vector.tensor_scalar_mul(attT[:], expv[:], rz[:])

                    # att = attT.T via 4 PE transposes
                    tp1 = p0tp.tile([128, 128], F32R, tag="tpa", name="tp1")
                    nc.tensor.transpose(tp1[:], attT0[:, 0:128], identr[:])
                    nc.vector.tensor_copy(att0[:, 0:128], tp1[:])
                    tp2 = p0tp.tile([128, 128], F32R, tag="tpa", name="tp2")
                    nc.tensor.transpose(tp2[:, 0:64], attT1[:, 0:128], identr[0:64, 0:64])
                    nc.vector.tensor_copy(att0[:, 128:192], tp2[:, 0:64])
                    tp3 = p0tp.tile([128, 128], F32R, tag="tpa", name="tp3")
                    nc.tensor.transpose(tp3[:64, :], attT0[:, 128:192], identr[:])
                    nc.vector.tensor_copy(att1[:, 0:128], tp3[:64, :])
                    tp4 = p0tp.tile([128, 128], F32R, tag="tpa", name="tp4")
                    nc.tensor.transpose(tp4[:64, 0:64], attT1[:, 128:192], identr[0:64, 0:64])
                    nc.vector.tensor_copy(att1[:, 128:192], tp4[:64, 0:64])

            # ---------------- conv: x2c = W * patches(x2T) ------------------
            # x2c[o, m] = sum_{kh,kw,c} convw[c, khw, o] * x2T[c, n(m,kh,kw)]
            with tc.tile_pool(name="cvw", bufs=1) as cvw, \
                 tc.tile_pool(name="cvp", bufs=2, space="PSUM") as cvp, \
                 tc.tile_pool(name="cvtp", bufs=2, space="PSUM") as cvtp:
                convw0 = cvw.tile([128, 16, D], F32R, tag="convw0", name="convw0")
                nc.sync.dma_start(convw0[:], convw.ap()[0:128])
                # convw1 duplicated on both partition halves so its base
                # partition matches the packed x2T1p slice it contracts with
                convw1 = cvw.tile([128, 16, D], F32R, tag="convw1", name="convw1")
                nc.sync.dma_start(convw1[0:64], convw.ap()[128:192])
                nc.sync.dma_start(convw1[64:128], convw.ap()[128:192])

                x2T0v = x2T0[:].rearrange(
                    "p (a i kh j kw) -> p a i kh j kw", a=2, i=16, kh=4, j=32, kw=4
                )
                x2T1v = x2T1p[:].rearrange(
                    "p (i kh j kw) -> p i kh j kw", i=16, kh=4, j=32, kw=4
                )
                for mh in range(2):  # m-halves of 512
                    pc0 = cvp.tile([128, 512], F32, tag="pc0", name="pc0")
                    pc1 = cvp.tile([64, 512], F32, tag="pc1", name="pc1")
                    for khw in range(16):
                        kh, kw = khw // 4, khw % 4
                        rhs0 = x2T0v[:, mh, :, kh, :, kw]
                        rhs1 = x2T1v[64 * mh : 64 * mh + 64, :, kh, :, kw]
                        for (ps, osl) in ((pc0, slice(0, 128)), (pc1, slice(128, 192))):
                            nc.tensor.matmul(
                                ps[:], convw0[:, khw, osl], rhs0,
                                start=(khw == 0), stop=False,
                            )
                            nc.tensor.matmul(
                                ps[:],
                                convw1[64 * mh : 64 * mh + 64, khw, osl],
                                rhs1,
                                start=False, stop=(khw == 15),
                            )
                    nc.scalar.activation(
                        x2c0[:, 512 * mh : 512 * (mh + 1)], pc0[:], AF.Identity,
                        bias=convb_sb[:, 0:1],
                    )
                    nc.scalar.activation(
                        x2c1[:, 512 * mh : 512 * (mh + 1)], pc1[:], AF.Identity,
                        bias=convb_sb[0:64, 1:2],
                    )
                # x2cT[m, mc, o]: transposes of x2c
                for mc in range(8):
                    tpc = cvtp.tile([128, 128], F32R, tag="tpc", name="tpc")
                    nc.tensor.transpose(
                        tpc[:], x2c0[:, 128 * mc : 128 * (mc + 1)], identr[:]
                    )
                    nc.vector.tensor_copy(x2cT[:, mc, 0:128], tpc[:])
                    tpd = cvtp.tile([128, 64], F32R, tag="tpd", name="tpd")
                    nc.tensor.transpose(
                        tpd[:], x2c1[:, 128 * mc : 128 * (mc + 1)], identr[0:64, 0:64]
                    )
                    nc.vector.tensor_copy(x2cT[:, mc, 128:192], tpd[:])

            # ---------------- main loop over n-blocks of the owned half -----
            with tc.tile_pool(name="mgt", bufs=1) as mgt, \
                 tc.tile_pool(name="mx1", bufs=2) as mx1, \
                 tc.tile_pool(name="mcat", bufs=2) as mcat, \
                 tc.tile_pool(name="msq", bufs=1) as msq, \
                 tc.tile_pool(name="mout", bufs=2) as mout, \
                 tc.tile_pool(name="mrows", bufs=2) as mrows, \
                 tc.tile_pool(name="pg", bufs=2, space="PSUM") as pg, \
                 tc.tile_pool(name="pcat", bufs=1, space="PSUM") as pcat, \
                 tc.tile_pool(name="pmisc", bufs=2, space="PSUM") as pmisc:
                for blk in range(NBLOCKS):
                    nb = 512 * blk  # offset within the half
                    ng = nb  # global n offset (owned half is rows 0:NH)

                    # x1T for this block: DMA natural rows, PE-transpose
                    xb = mx1.tile([128, 4, D], F32R, tag="xb", name="xb")
                    nc.sync.dma_start(
                        xb[:],
                        x_full.ap()[ng : ng + 512, 0:D].rearrange(
                            "(t p) c -> p t c", p=128
                        ),
                    )
                    x1t0 = mx1.tile([128, 512], F32R, tag="x1t0", name="x1t0")
                    x1t1 = mx1.tile([64, 512], F32R, tag="x1t1", name="x1t1")
                    for t in range(4):
                        tpe = pmisc.tile([128, 512], F32R, tag="pm", name="tpe")
                        nc.tensor.transpose(
                            tpe[:, 0:128], xb[:, t, 0:128], identr[:]
                        )
                        nc.vector.tensor_copy(
                            x1t0[:, 128 * t : 128 * (t + 1)], tpe[:, 0:128]
                        )
                        tpf = pmisc.tile([64, 512], F32R, tag="pm", name="tpf")
                        nc.tensor.transpose(tpf[:, 0:128], xb[:, t, 128:192], identr[:])
                        nc.vector.tensor_copy(
                            x1t1[:, 128 * t : 128 * (t + 1)], tpf[:, 0:128]
                        )

                    # gateT: 8 m-chunks of [128, 512]
                    gt = mgt.tile([128, 8, 512], F32R, tag="gt", name="gt")
                    for mc in range(8):
                        psg = pg.tile([128, 512], F32, tag="pg", name="psg")
                        nc.tensor.matmul(
                            psg[:], x2c0[:, 128 * mc : 128 * (mc + 1)], x1t0[:],
                            start=True, stop=False,
                        )
                        nc.tensor.matmul(
                            psg[:], x2c1[:, 128 * mc : 128 * (mc + 1)], x1t1[:],
                            start=False, stop=True,
                        )
                        nc.scalar.activation(gt[:, mc, :], psg[:], AF.Sigmoid)

                    # spT (d-chunks 128 + 64) and chT (e-chunks 64 + 128)
                    ps_sp0 = pcat.tile([128, 512], F32, tag="sp0", name="ps_sp0")
                    for mc in range(8):
                        nc.tensor.matmul(
                            ps_sp0[:], x2cT[:, mc, 0:128], gt[:, mc, :],
                            start=(mc == 0), stop=(mc == 7),
                        )
                    ps_sp1 = pcat.tile([64, 512], F32, tag="sp1", name="ps_sp1")
                    for mc in range(8):
                        nc.tensor.matmul(
                            ps_sp1[:], x2cT[:, mc, 128:192], gt[:, mc, :],
                            start=(mc == 0), stop=(mc == 7),
                        )
                    # chT e 0:64 -> cat rows 64:128 of chunk1
                    ps_cha = pcat.tile([64, 512], F32, tag="cha", name="ps_cha")
                    nc.tensor.matmul(
                        ps_cha[:], att0[:, 0:64],
                        x2T0[:, ng : ng + 512],
                        start=True, stop=False,
                    )
                    nc.tensor.matmul(
                        ps_cha[:], att1[:, 0:64],
                        x2T1p[0:64, nb : nb + 512],
                        start=False, stop=True,
                    )
                    ps_chb = pcat.tile([128, 512], F32, tag="chb", name="ps_chb")
                    nc.tensor.matmul(
                        ps_chb[:], att0[:, 64:192], x2T0[:, ng : ng + 512],
                        start=True, stop=False,
                    )
                    nc.tensor.matmul(
                        ps_chb[:], att1[:, 64:192],
                        x2T1p[0:64, nb : nb + 512],
                        start=False, stop=True,
                    )

                    # catT raw + squares to SBUF
                    cat = mcat.tile([128, 3, 512], F32R, tag="cat", name="cat")
                    sq = msq.tile([128, 3, 512], F32R, tag="sq", name="sq")
                    for (ps, k, rows) in (
                        (ps_sp0, 0, slice(0, 128)),
                        (ps_sp1, 1, slice(0, 64)),
                        (ps_cha, 1, slice(64, 128)),
                        (ps_chb, 2, slice(0, 128)),
                    ):
                        nc.vector.tensor_copy(cat[rows, k, :], ps[:])
                        nc.scalar.activation(sq[rows, k, :], ps[:], AF.Square)

                    # stats: mu, rstd rows
                    ps_s1 = pmisc.tile([1, 512], F32, tag="pm", name="ps_s1")
                    for k in range(3):
                        nc.tensor.matmul(
                            ps_s1[:], ones_col[:], cat[:, k, :],
                            start=(k == 0), stop=(k == 2),
                        )
                    # stats rows; rowA/rowC recycled in-place to cap SBUF
                    mu_row = mrows.tile([1, 512], F32, tag="rowA", name="mu_row")
                    nc.vector.tensor_scalar_mul(mu_row[:], ps_s1[:], 1.0 / C)
                    mu_r = mrows.tile([1, 512], F32R, tag="rowB", name="mu_r")
                    nc.vector.tensor_copy(mu_r[:], mu_row[:])
                    musq = mrows.tile([1, 512], F32, tag="rowC", name="musq")
                    nc.scalar.activation(musq[:], mu_row[:], AF.Square)
                    ps_s2 = pmisc.tile([1, 512], F32, tag="pm", name="ps_s2")
                    for k in range(3):
                        nc.tensor.matmul(
                            ps_s2[:], ones_col[:], sq[:, k, :],
                            start=(k == 0), stop=(k == 2),
                        )
                    e2_row = mrows.tile([1, 512], F32, tag="rowA", name="e2_row")
                    nc.vector.tensor_scalar_mul(e2_row[:], ps_s2[:], 1.0 / C)
                    # var -> std -> rstd, in place in musq's tile
                    nc.vector.tensor_tensor(musq[:], e2_row[:], musq[:], OP.subtract)
                    nc.scalar.activation(musq[:], musq[:], AF.Sqrt, bias=eps_sb[:])
                    nc.vector.reciprocal(musq[:], musq[:])

                    # rstd broadcast row -> all partitions (GPSIMD; frees a
                    # PSUM bank and keeps the final multiply SBUF-side)
                    rstd_bc = mout.tile([128, 512], F32, tag="rstd_bc", name="rstd_bc")
                    nc.gpsimd.partition_broadcast(rstd_bc[:], musq[:])

                    # proj: out = (pwTs.T @ catT - pwsum x mu) * rstd + bias2
                    for oc in range(3):
                        pso = pg.tile([128, 512], F32, tag="pg", name="pso")
                        for k in range(3):
                            nc.tensor.matmul(
                                pso[:],
                                pwTs[k][:, 128 * oc : 128 * (oc + 1)],
                                cat[:, k, :],
                                start=(k == 0), stop=False,
                            )
                        nc.tensor.matmul(
                            pso[:],
                            pwsumneg_row[:, 128 * oc : 128 * (oc + 1)],
                            mu_r[:],
                            start=False, stop=True,
                        )
                        osb = mout.tile([128, 512], F32, tag="osb", name="osb")
                        nc.vector.tensor_tensor(osb[:], pso[:], rstd_bc[:], OP.mult)
                        ofin = mout.tile([128, 512], F32, tag="ofin", name="ofin")
                        nc.scalar.activation(
                            ofin[:], osb[:], AF.Identity, bias=bias2_sb[:, oc : oc + 1]
                        )
                        nc.sync.dma_start(
                            out_part.ap()[128 * oc : 128 * (oc + 1), nb : nb + 512],
                            ofin[:],
                        )

    nc.finalize()
    return nc


_NC_CACHE: dict = {}


def _get_nc():
    if "nc" not in _NC_CACHE:
        _NC_CACHE["nc"] = build_nc()
    return _NC_CACHE["nc"]


def _prep_in_map(x_b, conv_w, conv_b, ln_w, ln_b, proj_w, proj_b):
    convw_t = np.ascontiguousarray(conv_w.transpose(1, 2, 3, 0)).reshape(D, 16, D)
    pwT = np.ascontiguousarray(proj_w.T)
    return {
        "x_full": np.ascontiguousarray(x_b, dtype=np.float32),
        "convw": convw_t.astype(np.float32),
        "convb": np.asarray(conv_b, dtype=np.float32),
        "lnw": np.asarray(ln_w, dtype=np.float32),
        "lnb": np.asarray(ln_b, dtype=np.float32),
        "pwT": pwT.astype(np.float32),
        "pb": np.asarray(proj_b, dtype=np.float32),
    }


def kernel(x, conv_w, conv_b, ln_w, ln_b, proj_w, proj_b, H=128, W=128):
    """Full-input entry point: shards over 8 cores (4 samples x 2 N-halves),
    runs the Bass kernel, gathers the full [B, C, N] output."""
    from concourse.bass_utils import run_bass_kernel_spmd

    x = np.asarray(x)
    assert x.shape == (B, N, C), x.shape

    # Core 2b + h handles (sample b, N-half h). Half-1 cores get the two
    # N-halves of x swapped so every core computes "rows 0:8192".
    nc = _get_nc()
    in_maps = []
    for b in range(B):
        for half in (0, 1):
            xb = x[b] if half == 0 else np.concatenate(
                [x[b, NH:], x[b, :NH]], axis=0
            )
            in_maps.append(
                _prep_in_map(xb, np.asarray(conv_w), np.asarray(conv_b),
                             np.asarray(ln_w), np.asarray(ln_b),
                             np.asarray(proj_w), np.asarray(proj_b))
            )
    res = run_bass_kernel_spmd(nc, in_maps, core_ids=list(range(8)))

    out = np.empty((B, C, N), dtype=np.float32)
    for b in range(B):
        for half in (0, 1):
            out[b][:, half * NH : (half + 1) * NH] = \
                res.results[2 * b + half]["out_part"]
    return out


# revision 11
# speedup vs baseline: 6.2940x; 6.2940x over previous
"""Trainium2 Bass kernel for nn_CSB (dense_transformer).

Reference computation (per sample b of B=4, N=16384, C=384, d=192, H=W=128,
M=N/16=1024):
  x1 = x[..., :d]; x2 = x[..., d:]
  x2c  = conv4x4s4(x2 as [d,H,W]) + conv_b            # [d, M]
  gate = sigmoid(x1 @ x2c)                            # [N, M]
  sp   = gate @ x2c.T                                 # [N, d]
  att  = softmax(x1.T @ x2, axis over first d)        # [d, d]
  ch   = x2 @ att                                     # [N, d]
  cat  = [sp, ch]; ln = LN(cat) * ln_w + ln_b
  out  = (ln @ proj_w.T + proj_b).T                   # [C, N]

Sharding: 8 cores = 4 samples x 2 N-halves. Each core takes the FULL sample
x[b] (s-matrix and conv need all N; computed redundantly in each half-pair)
and produces out[b][:, half*8192:(half+1)*8192].

All GEMMs run in float32r (fp32 storage, ~13-bit-mantissa multiply at full
PE rate for moving free-dim >= 256); PSUM accumulation is fp32.

Layout strategy (everything transposed, n on the free axis):
  - x2T (channels-major x2) built on-chip via PE transposes; feeds the conv
    (as strided patch views), chT, and (x1T per block) the gate.
  - gateT[m,n] = sigmoid(x2c.T @ x1T); spT[d,n] = x2cT.T @ gateT;
    chT[e,n] = att.T(d-major) @ x2T  ->  catT in [c,n] layout.
  - LN over partitions via ones-vector matmuls (sum / sum-of-squares),
    folded into the projection:
      out = (pwTs.T @ catT - pwsum x mu) * rstd_bc + bias2
    with pwTs = proj_w.T scaled by ln_w, bias2 = proj_w @ ln_b + proj_b.
"""

import sys
import types

_m = types.ModuleType("antenv.axon_hooks")
_m.get_axon_ntff_profile_hook = lambda: None
sys.modules.setdefault("antenv.axon_hooks", _m)

import numpy as np

import concourse.bacc as bacc
import concourse.mybir as mybir
import concourse.tile as tile
from concourse.masks import make_identity

F32 = mybir.dt.float32
F32R = mybir.dt.float32r
AF = mybir.ActivationFunctionType
OP = mybir.AluOpType

B = 4
N = 16384
C = 384
D = 192  # C // 2
M = 1024  # N // 16
NH = 8192  # N // 2, rows per core
NBLK = 512  # n-columns per main-loop block
NBLOCKS = NH // NBLK  # 16
EPS = 1e-5


def build_nc():
    """Build the per-core program. Every core "owns" rows 0:8192 of its
    x_full; cores handling the second N-half receive x_full with the two
    halves swapped (the s-matrix/attention are row-permutation invariant,
    and the conv's induced m-permutation cancels inside sp = sum_m gate*x2c,
    so the computed rows are exactly the owned rows)."""
    nc = bacc.Bacc(None, target_bir_lowering=False)

    x_full = nc.dram_tensor("x_full", [N, C], F32R, kind="ExternalInput")
    convw = nc.dram_tensor("convw", [D, 16, D], F32R, kind="ExternalInput")
    convb = nc.dram_tensor("convb", [D], F32, kind="ExternalInput")
    lnw_d = nc.dram_tensor("lnw", [C], F32, kind="ExternalInput")
    lnb_d = nc.dram_tensor("lnb", [C], F32R, kind="ExternalInput")
    pwT_d = nc.dram_tensor("pwT", [C, C], F32R, kind="ExternalInput")
    pb_d = nc.dram_tensor("pb", [C], F32, kind="ExternalInput")
    out_part = nc.dram_tensor("out_part", [C, NH], F32, kind="ExternalOutput")


    with tile.TileContext(nc) as tc:
        import contextlib

        with contextlib.ExitStack() as top:
            const = top.enter_context(tc.tile_pool(name="const", bufs=1))
            big = top.enter_context(tc.tile_pool(name="big", bufs=1))

            # ---------------- constants ----------------
            ident_f = const.tile([128, 128], F32, tag="ident_f")
            make_identity(nc, ident_f[:])
            identr = const.tile([128, 128], F32R, tag="identr")
            nc.vector.tensor_copy(identr[:], ident_f[:])

            ones_f = const.tile([128, 1], F32, tag="ones_f")
            nc.gpsimd.memset(ones_f[:], 1.0)
            ones_col = const.tile([128, 1], F32R, tag="ones_col")
            nc.vector.tensor_copy(ones_col[:], ones_f[:])
            onesr_f = const.tile([1, 128], F32, tag="onesr_f")
            nc.gpsimd.memset(onesr_f[:], 1.0)
            eps_sb = const.tile([1, 1], F32, tag="eps_sb")
            nc.gpsimd.memset(eps_sb[:], EPS)

            # per-channel vectors as [128, k] column stacks
            lnw_sb = const.tile([128, 3], F32, tag="lnw_sb")
            nc.sync.dma_start(lnw_sb[:], lnw_d.ap().rearrange("(o p) -> p o", p=128))
            lnb_sb = const.tile([128, 3], F32R, tag="lnb_sb")
            nc.sync.dma_start(lnb_sb[:], lnb_d.ap().rearrange("(o p) -> p o", p=128))
            pb_sb = const.tile([128, 3], F32, tag="pb_sb")
            nc.sync.dma_start(pb_sb[:], pb_d.ap().rearrange("(o p) -> p o", p=128))
            convb_sb = const.tile([128, 2], F32, tag="convb_sb")
            nc.sync.dma_start(convb_sb[:, 0:1], convb.ap()[0:128, None])
            nc.sync.dma_start(convb_sb[0:64, 1:2], convb.ap()[128:192, None])

            # proj weights: pwT [c, o]; pwTs = pwT * ln_w[c]; bias2 = P@lnb + pb
            pwTs = [
                const.tile([128, C], F32R, tag=f"pwTs{i}", name=f"pwTs{i}")
                for i in range(3)
            ]
            with tc.tile_pool(name="pwload", bufs=1) as pwload, \
                 tc.tile_pool(name="pwpsum", bufs=1, space="PSUM") as pwpsum:
                pwt_raw = [
                    pwload.tile([128, C], F32R, tag=f"pwt{i}", name=f"pwt{i}")
                    for i in range(3)
                ]
                for i in range(3):
                    nc.sync.dma_start(
                        pwt_raw[i][:], pwT_d.ap()[128 * i : 128 * (i + 1), :]
                    )
                # bias2 = proj_w @ ln_b + proj_b  (per-o, [128, 3])
                bias2_sb = const.tile([128, 3], F32, tag="bias2_sb")
                for oc in range(3):
                    psb = pwpsum.tile([128, 1], F32, tag="psb", name="psb")
                    for i in range(3):
                        # tiny free dims violate fp32r ISA restrictions; run
                        # these one-time matmuls as plain fp32 (bitcast)
                        nc.tensor.matmul(
                            psb[:],
                            pwt_raw[i][:, 128 * oc : 128 * (oc + 1)].bitcast(F32),
                            lnb_sb[:, i : i + 1].bitcast(F32),
                            start=(i == 0),
                            stop=(i == 2),
                        )
                    nc.scalar.activation(
                        bias2_sb[:, oc : oc + 1], psb[:], AF.Identity,
                        bias=pb_sb[:, oc : oc + 1],
                    )
                # pwTs = pwt * lnw (per-partition scalar on c)
                for i in range(3):
                    nc.vector.tensor_scalar_mul(
                        pwTs[i][:], pwt_raw[i][:], lnw_sb[:, i : i + 1]
                    )
                # pwsumneg_row[1, C] = -sum_c pwTs[c, o]
                pwsumneg_row = const.tile([1, C], F32R, tag="pwsumneg_row")
                pssum = pwpsum.tile([1, C], F32, tag="pssum", name="pssum")
                for i in range(3):
                    nc.tensor.matmul(
                        pssum[:], ones_f[:], pwTs[i][:].bitcast(F32),
                        start=(i == 0), stop=(i == 2),
                    )
                nc.vector.tensor_scalar_mul(pwsumneg_row[:], pssum[:], -1.0)

            # ---------------- resident big tensors ----------------
            # x2T0: [128, N]   channels 0:128 of x2, all N columns
            # x2T1p: [128, NH] channels 128:192 of x2; n-half j lives on
            #        partitions 64j:64j+64
            x2T0 = big.tile([128, N], F32R, tag="x2T0")
            x2T1p = big.tile([128, NH], F32R, tag="x2T1p")
            x2c0 = big.tile([128, M], F32R, tag="x2c0")  # conv out, o 0:128
            x2c1 = big.tile([64, M], F32R, tag="x2c1")  # conv out, o 128:192
            x2cT = big.tile([128, 8, D], F32R, tag="x2cT")  # [m-chunk, mc, o]
            att0 = big.tile([128, D], F32R, tag="att0")  # att[d0:128, e]
            att1 = big.tile([64, D], F32R, tag="att1")  # att[d 128:192, e]
            attT0 = big.tile([128, D], F32R, tag="attT0")  # attT[e0:128, d]
            attT1 = big.tile([64, D], F32R, tag="attT1")

            # ---------------- phase 0: stream x, s-matmuls, x2 transposes ----
            with tc.tile_pool(name="p0xt", bufs=3) as p0xt, \
                 tc.tile_pool(name="p0tp", bufs=2, space="PSUM") as p0tp, \
                 tc.tile_pool(name="p0s", bufs=1, space="PSUM") as p0s:
                sT0 = p0s.tile([128, 256], F32, tag="sT0", name="sT0")
                sT1 = p0s.tile([64, 256], F32, tag="sT1", name="sT1")
                for q in range(32):  # quads of 4 n-tiles
                    xt = p0xt.tile([128, 4, C], F32R, tag="xt", name="xt")
                    nc.sync.dma_start(
                        xt[:],
                        x_full.ap()[512 * q : 512 * (q + 1), :].rearrange(
                            "(t p) c -> p t c", p=128
                        ),
                    )
                    for j in range(4):
                        t = 4 * q + j
                        xtj = xt[:, j, :]
                        # s-matrix partials: sT[e, d] += x2_tile.T @ x1_tile
                        nc.tensor.matmul(
                            sT0[:], xtj[:, 192:320], xtj[:, 0:256],
                            start=(t == 0), stop=(t == N // 128 - 1),
                        )
                        nc.tensor.matmul(
                            sT1[:], xtj[:, 320:384], xtj[:, 0:256],
                            start=(t == 0), stop=(t == N // 128 - 1),
                        )
                        # x2 transposes into resident x2T
                        tpa = p0tp.tile([128, 128], F32R, tag="tpa", name="tpa")
                        nc.tensor.transpose(tpa[:], xtj[:, 192:320], identr[:])
                        nc.vector.tensor_copy(
                            x2T0[:, 128 * t : 128 * (t + 1)], tpa[:]
                        )
                        tpb = p0tp.tile([64, 128], F32R, tag="tpb", name="tpb")
                        nc.tensor.transpose(tpb[:], xtj[:, 320:384], identr[:])
                        nhalf_t = t // 64  # which n-half this tile is in
                        nc.vector.tensor_copy(
                            x2T1p[
                                64 * nhalf_t : 64 * nhalf_t + 64,
                                128 * (t % 64) : 128 * (t % 64) + 128,
                            ],
                            tpb[:],
                        )

                # ---------------- softmax over d (free axis of sT) ----------
                with tc.tile_pool(name="smx", bufs=1) as smx:
                    for (sps, ep, attT) in ((sT0, 128, attT0), (sT1, 64, attT1)):
                        mxn = smx.tile([ep, 1], F32, tag="mxn", name="mxn")
                        nc.vector.tensor_reduce(
                            mxn[:], sps[:ep, 0:D], mybir.AxisListType.X,
                            OP.max, negate=True,
                        )
                        expv = smx.tile([ep, D], F32R, tag="expv", name="expv")
                        nc.scalar.activation(
                            expv[:], sps[:ep, 0:D], AF.Exp, bias=mxn[:],
                        )
                        z = smx.tile([ep, 1], F32, tag="z", name="z")
                        nc.vector.reduce_sum(z[:], expv[:], axis=mybir.AxisListType.X)
                        rz = smx.tile([ep, 1], F32, tag="rz", name="rz")
                        nc.vector.reciprocal(rz[:], z[:])
                        nc.vector.tensor_scalar_mul(attT[:], expv[:], rz[:])

                    # att = attT.T via 4 PE transposes
                    tp1 = p0tp.tile([128, 128], F32R, tag="tpa", name="tp1")
                    nc.tensor.transpose(tp1[:], attT0[:, 0:128], identr[:])
                    nc.vector.tensor_copy(att0[:, 0:128], tp1[:])
                    tp2 = p0tp.tile([128, 128], F32R, tag="tpa", name="tp2")
                    nc.tensor.transpose(tp2[:, 0:64], attT1[:, 0:128], identr[0:64, 0:64])
                    nc.vector.tensor_copy(att0[:, 128:192], tp2[:, 0:64])
                    tp3 = p0tp.tile([128, 128], F32R, tag="tpa", name="tp3")
                    nc.tensor.transpose(tp3[:64, :], attT0[:, 128:192], identr[:])
                    nc.vector.tensor_copy(att1[:, 0:128], tp3[:64, :])
                    tp4 = p0tp.tile([128, 128], F32R, tag="tpa", name="tp4")
                    nc.tensor.transpose(tp4[:64, 0:64], attT1[:, 128:192], identr[0:64, 0:64])
                    nc.vector.tensor_copy(att1[:, 128:192], tp4[:64, 0:64])

            # ---------------- conv: x2c = W * patches(x2T) ------------------
            # x2c[o, m] = sum_{kh,kw,c} convw[c, khw, o] * x2T[c, n(m,kh,kw)]
            with tc.tile_pool(name="cvw", bufs=1) as cvw, \
                 tc.tile_pool(name="cvp", bufs=2, space="PSUM") as cvp, \
                 tc.tile_pool(name="cvtp", bufs=2, space="PSUM") as cvtp:
                convw0 = cvw.tile([128, 16, D], F32R, tag="convw0", name="convw0")
                nc.sync.dma_start(convw0[:], convw.ap()[0:128])
                # convw1 duplicated on both partition halves so its base
                # partition matches the packed x2T1p slice it contracts with
                convw1 = cvw.tile([128, 16, D], F32R, tag="convw1", name="convw1")
                nc.sync.dma_start(convw1[0:64], convw.ap()[128:192])
                nc.sync.dma_start(convw1[64:128], convw.ap()[128:192])

                x2T0v = x2T0[:].rearrange(
                    "p (a i kh j kw) -> p a i kh j kw", a=2, i=16, kh=4, j=32, kw=4
                )
                x2T1v = x2T1p[:].rearrange(
                    "p (i kh j kw) -> p i kh j kw", i=16, kh=4, j=32, kw=4
                )
                for mh in range(2):  # m-halves of 512
                    pc0 = cvp.tile([128, 512], F32, tag="pc0", name="pc0")
                    pc1 = cvp.tile([64, 512], F32, tag="pc1", name="pc1")
                    for khw in range(16):
                        kh, kw = khw // 4, khw % 4
                        rhs0 = x2T0v[:, mh, :, kh, :, kw]
                        rhs1 = x2T1v[64 * mh : 64 * mh + 64, :, kh, :, kw]
                        for (ps, osl) in ((pc0, slice(0, 128)), (pc1, slice(128, 192))):
                            nc.tensor.matmul(
                                ps[:], convw0[:, khw, osl], rhs0,
                                start=(khw == 0), stop=False,
                            )
                            nc.tensor.matmul(
                                ps[:],
                                convw1[64 * mh : 64 * mh + 64, khw, osl],
                                rhs1,
                                start=False, stop=(khw == 15),
                            )
                    nc.scalar.activation(
                        x2c0[:, 512 * mh : 512 * (mh + 1)], pc0[:], AF.Identity,
                        bias=convb_sb[:, 0:1],
                    )
                    nc.scalar.activation(
                        x2c1[:, 512 * mh : 512 * (mh + 1)], pc1[:], AF.Identity,
                        bias=convb_sb[0:64, 1:2],
                    )
                # x2cT[m, mc, o]: transposes of x2c
                for mc in range(8):
                    tpc = cvtp.tile([128, 128], F32R, tag="tpc", name="tpc")
                    nc.tensor.transpose(
                        tpc[:], x2c0[:, 128 * mc : 128 * (mc + 1)], identr[:]
                    )
                    nc.vector.tensor_copy(x2cT[:, mc, 0:128], tpc[:])
                    tpd = cvtp.tile([128, 64], F32R, tag="tpd", name="tpd")
                    nc.tensor.transpose(
                        tpd[:], x2c1[:, 128 * mc : 128 * (mc + 1)], identr[0:64, 0:64]
                    )
                    nc.vector.tensor_copy(x2cT[:, mc, 128:192], tpd[:])

            # ---------------- main loop over n-blocks of the owned half -----
            with tc.tile_pool(name="mgt", bufs=1) as mgt, \
                 tc.tile_pool(name="mx1", bufs=2) as mx1, \
                 tc.tile_pool(name="mcat", bufs=2) as mcat, \
                 tc.tile_pool(name="msq", bufs=1) as msq, \
                 tc.tile_pool(name="mout", bufs=2) as mout, \
                 tc.tile_pool(name="mrows", bufs=2) as mrows, \
                 tc.tile_pool(name="pg", bufs=2, space="PSUM") as pg, \
                 tc.tile_pool(name="pcat", bufs=1, space="PSUM") as pcat, \
                 tc.tile_pool(name="pmisc", bufs=2, space="PSUM") as pmisc:
                for blk in range(NBLOCKS):
                    nb = 512 * blk  # offset within the half
                    ng = nb  # global n offset (owned half is rows 0:NH)

                    # x1T for this block: DMA natural rows, PE-transpose
                    xb = mx1.tile([128, 4, D], F32R, tag="xb", name="xb")
                    nc.sync.dma_start(
                        xb[:],
                        x_full.ap()[ng : ng + 512, 0:D].rearrange(
                            "(t p) c -> p t c", p=128
                        ),
                    )
                    x1t0 = mx1.tile([128, 512], F32R, tag="x1t0", name="x1t0")
                    x1t1 = mx1.tile([64, 512], F32R, tag="x1t1", name="x1t1")
                    for t in range(4):
                        tpe = pmisc.tile([128, 512], F32R, tag="pm", name="tpe")
                        nc.tensor.transpose(
                            tpe[:, 0:128], xb[:, t, 0:128], identr[:]
                        )
                        nc.vector.tensor_copy(
                            x1t0[:, 128 * t : 128 * (t + 1)], tpe[:, 0:128]
                        )
                        tpf = pmisc.tile([64, 512], F32R, tag="pm", name="tpf")
                        nc.tensor.transpose(tpf[:, 0:128], xb[:, t, 128:192], identr[:])
                        nc.vector.tensor_copy(
                            x1t1[:, 128 * t : 128 * (t + 1)], tpf[:, 0:128]
                        )

                    # gateT: 8 m-chunks of [128, 512]
                    gt = mgt.tile([128, 8, 512], F32R, tag="gt", name="gt")
                    for mc in range(8):
                        psg = pg.tile([128, 512], F32, tag="pg", name="psg")
                        nc.tensor.matmul(
                            psg[:], x2c0[:, 128 * mc : 128 * (mc + 1)], x1t0[:],
                            start=True, stop=False,
                        )
                        nc.tensor.matmul(
                            psg[:], x2c1[:, 128 * mc : 128 * (mc + 1)], x1t1[:],
                            start=False, stop=True,
                        )
                        nc.scalar.activation(gt[:, mc, :], psg[:], AF.Sigmoid)

                    # spT (d-chunks 128 + 64) and chT (e-chunks 64 + 128)
                    ps_sp0 = pcat.tile([128, 512], F32, tag="sp0", name="ps_sp0")
                    for mc in range(8):
                        nc.tensor.matmul(
                            ps_sp0[:], x2cT[:, mc, 0:128], gt[:, mc, :],
                            start=(mc == 0), stop=(mc == 7),
                        )
                    ps_sp1 = pcat.tile([64, 512], F32, tag="sp1", name="ps_sp1")
                    for mc in range(8):
                        nc.tensor.matmul(
                            ps_sp1[:], x2cT[:, mc, 128:192], gt[:, mc, :],
                            start=(mc == 0), stop=(mc == 7),
                        )
                    # chT e 0:64 -> cat rows 64:128 of chunk1
                    ps_cha = pcat.tile([64, 512], F32, tag="cha", name="ps_cha")
                    nc.tensor.matmul(
                        ps_cha[:], att0[:, 0:64],
                        x2T0[:, ng : ng + 512],
                        start=True, stop=False,
                    )
                    nc.tensor.matmul(
                        ps_cha[:], att1[:, 0:64],
                        x2T1p[0:64, nb : nb + 512],
                        start=False, stop=True,
                    )
                    ps_chb = pcat.tile([128, 512], F32, tag="chb", name="ps_chb")
                    nc.tensor.matmul(
                        ps_chb[:], att0[:, 64:192], x2T0[:, ng : ng + 512],
                        start=True, stop=False,
                    )
                    nc.tensor.matmul(
                        ps_chb[:], att1[:, 64:192],
                        x2T1p[0:64, nb : nb + 512],
                        start=False, stop=True,
                    )

                    # catT raw + squares to SBUF
                    cat = mcat.tile([128, 3, 512], F32R, tag="cat", name="cat")
                    sq = msq.tile([128, 3, 512], F32R, tag="sq", name="sq")
                    for (ps, k, rows) in (
                        (ps_sp0, 0, slice(0, 128)),
                        (ps_sp1, 1, slice(0, 64)),
                        (ps_cha, 1, slice(64, 128)),
                        (ps_chb, 2, slice(0, 128)),
                    ):
                        nc.vector.tensor_copy(cat[rows, k, :], ps[:])
                        nc.scalar.activation(sq[rows, k, :], ps[:], AF.Square)

                    # stats: mu, rstd rows
                    ps_s1 = pmisc.tile([1, 512], F32, tag="pm", name="ps_s1")
                    for k in range(3):
                        nc.tensor.matmul(
                            ps_s1[:], ones_col[:], cat[:, k, :],
                            start=(k == 0), stop=(k == 2),
                        )
                    # stats rows; rowA/rowC recycled in-place to cap SBUF
                    mu_row = mrows.tile([1, 512], F32, tag="rowA", name="mu_row")
                    nc.vector.tensor_scalar_mul(mu_row[:], ps_s1[:], 1.0 / C)
                    mu_r = mrows.tile([1, 512], F32R, tag="rowB", name="mu_r")
                    nc.vector.tensor_copy(mu_r[:], mu_row[:])
                    musq = mrows.tile([1, 512], F32, tag="rowC", name="musq")
                    nc.scalar.activation(musq[:], mu_row[:], AF.Square)
                    ps_s2 = pmisc.tile([1, 512], F32, tag="pm", name="ps_s2")
                    for k in range(3):
                        nc.tensor.matmul(
                            ps_s2[:], ones_col[:], sq[:, k, :],
                            start=(k == 0), stop=(k == 2),
                        )
                    e2_row = mrows.tile([1, 512], F32, tag="rowA", name="e2_row")
                    nc.vector.tensor_scalar_mul(e2_row[:], ps_s2[:], 1.0 / C)
                    # var -> std -> rstd, in place in musq's tile
                    nc.vector.tensor_tensor(musq[:], e2_row[:], musq[:], OP.subtract)
                    nc.scalar.activation(musq[:], musq[:], AF.Sqrt, bias=eps_sb[:])
                    nc.vector.reciprocal(musq[:], musq[:])

                    # rstd broadcast row -> all partitions via fp32 rank-1
                    # matmul (ones x rstd), then to SBUF (DVE may read only one
                    # PSUM operand in the final multiply)
                    ps_bc = pmisc.tile([128, 512], F32, tag="pm", name="ps_bc")
                    nc.tensor.matmul(
                        ps_bc[:], onesr_f[:], musq[:], start=True, stop=True
                    )
                    rstd_bc = mout.tile([128, 512], F32, tag="rstd_bc", name="rstd_bc")
                    nc.scalar.activation(rstd_bc[:], ps_bc[:], AF.Copy)

                    # proj: out = (pwTs.T @ catT - pwsum x mu) * rstd + bias2
                    for oc in range(3):
                        pso = pg.tile([128, 512], F32, tag="pg", name="pso")
                        for k in range(3):
                            nc.tensor.matmul(
                                pso[:],
                                pwTs[k][:, 128 * oc : 128 * (oc + 1)],
                                cat[:, k, :],
                                start=(k == 0), stop=False,
                            )
                        nc.tensor.matmul(
                            pso[:],
                            pwsumneg_row[:, 128 * oc : 128 * (oc + 1)],
                            mu_r[:],
                            start=False, stop=True,
                        )
                        osb = mout.tile([128, 512], F32, tag="osb", name="osb")
                        nc.vector.tensor_tensor(osb[:], pso[:], rstd_bc[:], OP.mult)
                        ofin = mout.tile([128, 512], F32, tag="ofin", name="ofin")
                        nc.scalar.activation(
                            ofin[:], osb[:], AF.Identity, bias=bias2_sb[:, oc : oc + 1]
                        )
                        nc.sync.dma_start(
                            out_part.ap()[128 * oc : 128 * (oc + 1), nb : nb + 512],
                            ofin[:],
                        )

    nc.finalize()
    return nc


_NC_CACHE: dict = {}


def _get_nc():
    if "nc" not in _NC_CACHE:
        _NC_CACHE["nc"] = build_nc()
    return _NC_CACHE["nc"]


def _prep_in_map(x_b, conv_w, conv_b, ln_w, ln_b, proj_w, proj_b):
    convw_t = np.ascontiguousarray(conv_w.transpose(1, 2, 3, 0)).reshape(D, 16, D)
    pwT = np.ascontiguousarray(proj_w.T)
    return {
        "x_full": np.ascontiguousarray(x_b, dtype=np.float32),
        "convw": convw_t.astype(np.float32),
        "convb": np.asarray(conv_b, dtype=np.float32),
        "lnw": np.asarray(ln_w, dtype=np.float32),
        "lnb": np.asarray(ln_b, dtype=np.float32),
        "pwT": pwT.astype(np.float32),
        "pb": np.asarray(proj_b, dtype=np.float32),
    }


def kernel(x, conv_w, conv_b, ln_w, ln_b, proj_w, proj_b, H=128, W=128):
    """Full-input entry point: shards over 8 cores (4 samples x 2 N-halves),
    runs the Bass kernel, gathers the full [B, C, N] output."""
    from concourse.bass_utils import run_bass_kernel_spmd

    x = np.asarray(x)
    assert x.shape == (B, N, C), x.shape

    # Core 2b + h handles (sample b, N-half h). Half-1 cores get the two
    # N-halves of x swapped so every core computes "rows 0:8192".
    nc = _get_nc()
    in_maps = []
    for b in range(B):
        for half in (0, 1):
            xb = x[b] if half == 0 else np.concatenate(
                [x[b, NH:], x[b, :NH]], axis=0
            )
            in_maps.append(
                _prep_in_map(xb, np.asarray(conv_w), np.asarray(conv_b),
                             np.asarray(ln_w), np.asarray(ln_b),
                             np.asarray(proj_w), np.asarray(proj_b))
            )
    res = run_bass_kernel_spmd(nc, in_maps, core_ids=list(range(8)))

    out = np.empty((B, C, N), dtype=np.float32)
    for b in range(B):
        for half in (0, 1):
            out[b][:, half * NH : (half + 1) * NH] = \
                res.results[2 * b + half]["out_part"]
    return out


# revision 37
# speedup vs baseline: 298.2294x; 47.3834x over previous
"""Trainium2 Bass kernel for nn_CSB (dense_transformer).

Reference computation (per sample b of B=4, N=16384, C=384, d=192, H=W=128,
M=N/16=1024):
  x1 = x[..., :d]; x2 = x[..., d:]
  x2c  = conv4x4s4(x2 as [d,H,W]) + conv_b            # [d, M]
  gate = sigmoid(x1 @ x2c)                            # [N, M]
  sp   = gate @ x2c.T                                 # [N, d]
  att  = softmax(x1.T @ x2, axis over first d)        # [d, d]
  ch   = x2 @ att                                     # [N, d]
  cat  = [sp, ch]; ln = LN(cat) * ln_w + ln_b
  out  = (ln @ proj_w.T + proj_b).T                   # [C, N]

Sharding: 8 cores = 4 samples x 2 N-halves. Each core takes the FULL sample
x[b] (s-matrix and conv need all N; computed redundantly in each half-pair)
and produces out[b][:, half*8192:(half+1)*8192].

All GEMMs run in float32r (fp32 storage, ~13-bit-mantissa multiply at full
PE rate for moving free-dim >= 256); PSUM accumulation is fp32.

Layout strategy (everything transposed, n on the free axis):
  - x2T (channels-major x2) built on-chip via PE transposes; feeds the conv
    (as strided patch views), chT, and (x1T per block) the gate.
  - gateT[m,n] = sigmoid(x2c.T @ x1T); spT[d,n] = x2cT.T @ gateT;
    chT[e,n] = att.T(d-major) @ x2T  ->  catT in [c,n] layout.
  - LN over partitions via ones-vector matmuls (sum / sum-of-squares),
    folded into the projection:
      out = (pwTs.T @ catT - pwsum x mu) * rstd_bc + bias2
    with pwTs = proj_w.T scaled by ln_w, bias2 = proj_w @ ln_b + proj_b.
"""

import sys
import types

_m = types.ModuleType("antenv.axon_hooks")
_m.get_axon_ntff_profile_hook = lambda: None
sys.modules.setdefault("antenv.axon_hooks", _m)

import numpy as np

import concourse.bacc as bacc
import concourse.mybir as mybir
import concourse.tile as tile
from concourse.masks import make_identity

F32 = mybir.dt.float32
F32R = mybir.dt.float32r
AF = mybir.ActivationFunctionType
OP = mybir.AluOpType

B = 4
N = 16384
C = 384
D = 192  # C // 2
M = 1024  # N // 16
NH = 8192  # N // 2, rows per core
NBLK = 512  # n-columns per main-loop block
NBLOCKS = NH // NBLK  # 16
EPS = 1e-5


def build_nc():
    """Build the per-core program. Every core "owns" rows 0:8192 of its
    x_full; cores handling the second N-half receive x_full with the two
    halves swapped (the s-matrix/attention are row-permutation invariant,
    and the conv's induced m-permutation cancels inside sp = sum_m gate*x2c,
    so the computed rows are exactly the owned rows)."""
    nc = bacc.Bacc(None, target_bir_lowering=False)

    x_full = nc.dram_tensor("x_full", [N, C], F32R, kind="ExternalInput")
    convw = nc.dram_tensor("convw", [D, 16, D], F32R, kind="ExternalInput")
    convb = nc.dram_tensor("convb", [D], F32, kind="ExternalInput")
    lnw_d = nc.dram_tensor("lnw", [C], F32, kind="ExternalInput")
    lnb_d = nc.dram_tensor("lnb", [C], F32R, kind="ExternalInput")
    pwT_d = nc.dram_tensor("pwT", [C, C], F32R, kind="ExternalInput")
    pb_d = nc.dram_tensor("pb", [C], F32, kind="ExternalInput")
    out_part = nc.dram_tensor("out_part", [C, NH], F32, kind="ExternalOutput")


    with tile.TileContext(nc) as tc:
        import contextlib

        with contextlib.ExitStack() as top:
            const = top.enter_context(tc.tile_pool(name="const", bufs=1))
            big = top.enter_context(tc.tile_pool(name="big", bufs=1))

            # ---------------- constants ----------------
            ident_f = const.tile([128, 128], F32, tag="ident_f")
            make_identity(nc, ident_f[:])
            identr = const.tile([128, 128], F32R, tag="identr")
            nc.vector.tensor_copy(identr[:], ident_f[:])

            ones_f = const.tile([128, 1], F32, tag="ones_f")
            nc.gpsimd.memset(ones_f[:], 1.0)
            ones_col = const.tile([128, 1], F32R, tag="ones_col")
            nc.vector.tensor_copy(ones_col[:], ones_f[:])
            invC_f = const.tile([128, 1], F32, tag="invC_f")
            nc.gpsimd.memset(invC_f[:], 1.0 / C)
            invC_col = const.tile([128, 1], F32R, tag="invC_col")
            nc.vector.tensor_copy(invC_col[:], invC_f[:])
            onesr_f = const.tile([1, 128], F32, tag="onesr_f")
            nc.gpsimd.memset(onesr_f[:], 1.0)
            onesr_r = const.tile([1, 128], F32R, tag="onesr_r")
            nc.vector.tensor_copy(onesr_r[:], onesr_f[:])
            eps_sb = const.tile([1, 1], F32, tag="eps_sb")
            nc.gpsimd.memset(eps_sb[:], EPS)

            # per-channel vectors as [128, k] column stacks
            lnw_sb = const.tile([128, 3], F32, tag="lnw_sb")
            nc.sync.dma_start(lnw_sb[:], lnw_d.ap().rearrange("(o p) -> p o", p=128))
            lnb_sb = const.tile([128, 3], F32R, tag="lnb_sb")
            nc.sync.dma_start(lnb_sb[:], lnb_d.ap().rearrange("(o p) -> p o", p=128))
            pb_sb = const.tile([128, 3], F32, tag="pb_sb")
            nc.sync.dma_start(pb_sb[:], pb_d.ap().rearrange("(o p) -> p o", p=128))
            convb_sb = const.tile([128, 2], F32, tag="convb_sb")
            nc.sync.dma_start(convb_sb[:, 0:1], convb.ap()[0:128, None])
            nc.sync.dma_start(convb_sb[0:64, 1:2], convb.ap()[128:192, None])

            # proj weights: pwT [c, o]; pwTs = pwT * ln_w[c]; bias2 = P@lnb + pb
            pwTs = [
                const.tile([128, C], F32R, tag=f"pwTs{i}", name=f"pwTs{i}")
                for i in range(3)
            ]
            with tc.tile_pool(name="pwload", bufs=1) as pwload, \
                 tc.tile_pool(name="pwpsum", bufs=1, space="PSUM") as pwpsum:
                pwt_raw = [
                    pwload.tile([128, C], F32R, tag=f"pwt{i}", name=f"pwt{i}")
                    for i in range(3)
                ]
                for i in range(3):
                    nc.sync.dma_start(
                        pwt_raw[i][:], pwT_d.ap()[128 * i : 128 * (i + 1), :]
                    )
                # bias2 = proj_w @ ln_b + proj_b  (per-o, [128, 3])
                bias2_sb = const.tile([128, 3], F32, tag="bias2_sb")
                for oc in range(3):
                    psb = pwpsum.tile([128, 1], F32, tag="psb", name="psb")
                    for i in range(3):
                        # tiny free dims violate fp32r ISA restrictions; run
                        # these one-time matmuls as plain fp32 (bitcast)
                        nc.tensor.matmul(
                            psb[:],
                            pwt_raw[i][:, 128 * oc : 128 * (oc + 1)].bitcast(F32),
                            lnb_sb[:, i : i + 1].bitcast(F32),
                            start=(i == 0),
                            stop=(i == 2),
                        )
                    nc.scalar.activation(
                        bias2_sb[:, oc : oc + 1], psb[:], AF.Identity,
                        bias=pb_sb[:, oc : oc + 1],
                    )
                # pwTs = pwt * lnw (per-partition scalar on c)
                for i in range(3):
                    nc.vector.tensor_scalar_mul(
                        pwTs[i][:], pwt_raw[i][:], lnw_sb[:, i : i + 1]
                    )
                # pwsumneg_row[1, C] = -sum_c pwTs[c, o]
                pwsumneg_row = const.tile([1, C], F32R, tag="pwsumneg_row")
                pssum = pwpsum.tile([1, C], F32, tag="pssum", name="pssum")
                for i in range(3):
                    nc.tensor.matmul(
                        pssum[:], ones_f[:], pwTs[i][:].bitcast(F32),
                        start=(i == 0), stop=(i == 2),
                    )
                nc.vector.tensor_scalar_mul(pwsumneg_row[:], pssum[:], -1.0)

            # ---------------- resident big tensors ----------------
            # x2T0: [128, N]   channels 0:128 of x2, all N columns
            # x2T1p: [128, NH] channels 128:192 of x2; n-half j lives on
            #        partitions 64j:64j+64
            x2T0 = big.tile([128, N], F32R, tag="x2T0")
            x2T1p = big.tile([128, NH], F32R, tag="x2T1p")
            x2c0 = big.tile([128, M], F32R, tag="x2c0")  # conv out, o 0:128
            x2c1 = big.tile([64, M], F32R, tag="x2c1")  # conv out, o 128:192
            x2cT = big.tile([128, 8, D], F32R, tag="x2cT")  # [m-chunk, mc, o]
            att0 = big.tile([128, D], F32R, tag="att0")  # att[d0:128, e]
            att1 = big.tile([64, D], F32R, tag="att1")  # att[d 128:192, e]
            attT0 = big.tile([128, D], F32R, tag="attT0")  # attT[e0:128, d]
            attT1 = big.tile([64, D], F32R, tag="attT1")

            # ---------------- phase 0: stream x, s-matmuls, x2 transposes ----
            # The conv for m-half h only needs the x2T columns of n-half h,
            # so its matmuls are woven in right after each half's streaming
            # completes, hiding conv compute behind the DMA of the other half.
            with tc.tile_pool(name="p0xt", bufs=5) as p0xt, \
                 tc.tile_pool(name="p0tp", bufs=2, space="PSUM") as p0tp, \
                 tc.tile_pool(name="p0s", bufs=1, space="PSUM") as p0s, \
                 tc.tile_pool(name="cvw", bufs=1) as cvw, \
                 tc.tile_pool(name="cvp", bufs=1, space="PSUM") as cvp:
                sT0 = p0s.tile([128, 256], F32, tag="sT0", name="sT0")
                sT1 = p0s.tile([64, 256], F32, tag="sT1", name="sT1")
                convw0 = cvw.tile([128, 16, D], F32R, tag="convw0", name="convw0")
                convw1 = cvw.tile([128, 16, D], F32R, tag="convw1", name="convw1")

                def load_convw():
                    # deferred so the first stream quads win the DMA queues
                    nc.sync.dma_start(convw0[:], convw.ap()[0:128])
                    # convw1 duplicated on both partition halves so its base
                    # partition matches the packed x2T1p slice it contracts
                    nc.sync.dma_start(convw1[0:64], convw.ap()[128:192])
                    nc.sync.dma_start(convw1[64:128], convw.ap()[128:192])

                x2T0v = x2T0[:].rearrange(
                    "p (a i kh j kw) -> p a i kh j kw", a=2, i=16, kh=4, j=32, kw=4
                )
                x2T1v = x2T1p[:].rearrange(
                    "p (i kh j kw) -> p i kh j kw", i=16, kh=4, j=32, kw=4
                )

                def stream_quad(q):
                    xt = p0xt.tile([128, 4, C], F32R, tag="xt", name="xt")
                    nc.sync.dma_start(
                        xt[:],
                        x_full.ap()[512 * q : 512 * (q + 1), :].rearrange(
                            "(t p) c -> p t c", p=128
                        ),
                    )
                    # batched transpose psums: 4 tiles -> one eviction each
                    tpa = p0tp.tile([128, 512], F32R, tag="tpa", name="tpa")
                    tpb = p0tp.tile([64, 512], F32R, tag="tpb", name="tpb")
                    for j in range(4):
                        t = 4 * q + j
                        xtj = xt[:, j, :]
                        # s-matrix partials: sT[e, d] += x2_tile.T @ x1_tile
                        nc.tensor.matmul(
                            sT0[:], xtj[:, 192:320], xtj[:, 0:256],
                            start=(t == 0), stop=(t == N // 128 - 1),
                        )
                        nc.tensor.matmul(
                            sT1[:], xtj[:, 320:384], xtj[:, 0:256],
                            start=(t == 0), stop=(t == N // 128 - 1),
                        )
                        # x2 transposes into the batched psums
                        nc.tensor.transpose(
                            tpa[:, 128 * j : 128 * (j + 1)], xtj[:, 192:320],
                            identr[:],
                        )
                        nc.tensor.transpose(
                            tpb[:, 128 * j : 128 * (j + 1)], xtj[:, 320:384],
                            identr[:],
                        )
                    nc.vector.tensor_copy(
                        x2T0[:, 512 * q : 512 * (q + 1)], tpa[:]
                    )
                    nhalf_q = q // 16
                    qq = q % 16
                    nc.vector.tensor_copy(
                        x2T1p[64 * nhalf_q : 64 * nhalf_q + 64,
                              512 * qq : 512 * (qq + 1)],
                        tpb[:],
                    )

                def conv_quarter(qi):
                    # x2c[o, 256*qi : 256*(qi+1)]; needs only stream quads
                    # 8*qi .. 8*qi+7, so conv work starts at quad 8 and fills
                    # the DMA-bound streaming gaps
                    mh, ih = qi // 2, 8 * (qi % 2)
                    pc0 = cvp.tile([128, 256], F32, tag="pc0", name="pc0")
                    pc1 = cvp.tile([64, 256], F32, tag="pc1", name="pc1")
                    for khw in range(16):
                        kh, kw = khw // 4, khw % 4
                        rhs0 = x2T0v[:, mh, ih : ih + 8, kh, :, kw]
                        rhs1 = x2T1v[64 * mh : 64 * mh + 64, ih : ih + 8, kh, :, kw]
                        for (ps, osl) in ((pc0, slice(0, 128)), (pc1, slice(128, 192))):
                            nc.tensor.matmul(
                                ps[:], convw0[:, khw, osl], rhs0,
                                start=(khw == 0), stop=False,
                            )
                            nc.tensor.matmul(
                                ps[:],
                                convw1[64 * mh : 64 * mh + 64, khw, osl],
                                rhs1,
                                start=False, stop=(khw == 15),
                            )
                    mq = 256 * qi
                    nc.scalar.activation(
                        x2c0[:, mq : mq + 256], pc0[:], AF.Identity,
                        bias=convb_sb[:, 0:1],
                    )
                    nc.scalar.activation(
                        x2c1[:, mq : mq + 256], pc1[:], AF.Identity,
                        bias=convb_sb[0:64, 1:2],
                    )

                def x2cT_quarter(mc0):
                    # x2cT[m, mc, o] for m-chunks mc0..mc0+3
                    for mc in range(mc0, mc0 + 4):
                        tpc = p0tp.tile([128, 128], F32R, tag="tpa", name="tpc")
                        nc.tensor.transpose(
                            tpc[:], x2c0[:, 128 * mc : 128 * (mc + 1)], identr[:]
                        )
                        nc.vector.tensor_copy(x2cT[:, mc, 0:128], tpc[:])
                        tpd = p0tp.tile([128, 64], F32R, tag="tpb", name="tpd")
                        nc.tensor.transpose(
                            tpd[:], x2c1[:, 128 * mc : 128 * (mc + 1)],
                            identr[0:64, 0:64],
                        )
                        nc.vector.tensor_copy(x2cT[:, mc, 128:192], tpd[:])

                for q in range(32):
                    stream_quad(q)
                    if q == 2:
                        load_convw()
                    if q >= 8 and q % 8 == 7:
                        conv_quarter(q // 8 - 1)
                conv_quarter(3)

                # ---------------- softmax over d (free axis of sT) ----------
                with tc.tile_pool(name="smx", bufs=1) as smx:
                    for (sps, ep, attT) in ((sT0, 128, attT0), (sT1, 64, attT1)):
                        mxn = smx.tile([ep, 1], F32, tag="mxn", name="mxn")
                        nc.vector.tensor_reduce(
                            mxn[:], sps[:ep, 0:D], mybir.AxisListType.X,
                            OP.max, negate=True,
                        )
                        expv = smx.tile([ep, D], F32R, tag="expv", name="expv")
                        nc.scalar.activation(
                            expv[:], sps[:ep, 0:D], AF.Exp, bias=mxn[:],
                        )
                        z = smx.tile([ep, 1], F32, tag="z", name="z")
                        nc.vector.reduce_sum(z[:], expv[:], axis=mybir.AxisListType.X)
                        rz = smx.tile([ep, 1], F32, tag="rz", name="rz")
                        nc.vector.reciprocal(rz[:], z[:])
                        nc.vector.tensor_scalar_mul(attT[:], expv[:], rz[:])

                    x2cT_quarter(0)
                    x2cT_quarter(4)

                    # att = attT.T via 4 PE transposes
                    tp1 = p0tp.tile([128, 128], F32R, tag="tpa", name="tp1")
                    nc.tensor.transpose(tp1[:], attT0[:, 0:128], identr[:])
                    nc.vector.tensor_copy(att0[:, 0:128], tp1[:])
                    tp2 = p0tp.tile([128, 128], F32R, tag="tpa", name="tp2")
                    nc.tensor.transpose(tp2[:, 0:64], attT1[:, 0:128], identr[0:64, 0:64])
                    nc.vector.tensor_copy(att0[:, 128:192], tp2[:, 0:64])
                    tp3 = p0tp.tile([128, 128], F32R, tag="tpa", name="tp3")
                    nc.tensor.transpose(tp3[:64, :], attT0[:, 128:192], identr[:])
                    nc.vector.tensor_copy(att1[:, 0:128], tp3[:64, :])
                    tp4 = p0tp.tile([128, 128], F32R, tag="tpa", name="tp4")
                    nc.tensor.transpose(tp4[:64, 0:64], attT1[:, 128:192], identr[0:64, 0:64])
                    nc.vector.tensor_copy(att1[:, 128:192], tp4[:64, 0:64])

            # ---------------- main loop over n-blocks of the owned half -----
            with tc.tile_pool(name="mgt", bufs=1) as mgt, \
                 tc.tile_pool(name="mx1", bufs=2) as mx1, \
                 tc.tile_pool(name="mcat", bufs=3) as mcat, \
                 tc.tile_pool(name="msq", bufs=1) as msq, \
                 tc.tile_pool(name="mout", bufs=2) as mout, \
                 tc.tile_pool(name="mrows", bufs=2) as mrows, \
                 tc.tile_pool(name="pg", bufs=3, space="PSUM") as pg, \
                 tc.tile_pool(name="pwave", bufs=2, space="PSUM") as pwave, \
                 tc.tile_pool(name="pproj", bufs=2, space="PSUM") as pproj, \
                 tc.tile_pool(name="pmisc", bufs=1, space="PSUM") as pmisc:
                def make_x1t(blk):
                    # x1T for block blk: DMA natural rows, PE-transpose
                    ng = 512 * blk
                    xb = mx1.tile([128, 4, D], F32R, tag="xb", name="xb")
                    nc.sync.dma_start(
                        xb[:],
                        x_full.ap()[ng : ng + 512, 0:D].rearrange(
                            "(t p) c -> p t c", p=128
                        ),
                    )
                    x1t0 = mx1.tile([128, 512], F32R, tag="x1t0", name="x1t0")
                    x1t1 = mx1.tile([64, 512], F32R, tag="x1t1", name="x1t1")
                    tpe = pmisc.tile([128, 512], F32R, tag="pm", name="tpe")
                    tpf = pmisc.tile([64, 512], F32R, tag="pm", name="tpf")
                    for t in range(4):
                        nc.tensor.transpose(
                            tpe[:, 128 * t : 128 * (t + 1)], xb[:, t, 0:128],
                            identr[:],
                        )
                        nc.tensor.transpose(
                            tpf[:, 128 * t : 128 * (t + 1)], xb[:, t, 128:192],
                            identr[:],
                        )
                    nc.any.tensor_copy(x1t0[:], tpe[:])
                    nc.any.tensor_copy(x1t1[:], tpf[:])
                    return x1t0, x1t1

                next_x1t = {0: make_x1t(0)}

                def emit_front(blk):
                    """gates, ch, sp, evictions, stats matmuls for one block.
                    Returns handles needed by the tail."""
                    nb = 512 * blk
                    ng = nb
                    x1t0, x1t1 = next_x1t.pop(blk)

                    gt = mgt.tile([128, 8, 512], F32R, tag="gt", name="gt")
                    for mc in range(8):
                        psg = pg.tile([128, 512], F32, tag="pg", name="psg")
                        nc.tensor.matmul(
                            psg[:], x2c0[:, 128 * mc : 128 * (mc + 1)], x1t0[:],
                            start=True, stop=False,
                        )
                        nc.tensor.matmul(
                            psg[:], x2c1[:, 128 * mc : 128 * (mc + 1)], x1t1[:],
                            start=False, stop=True,
                        )
                        nc.scalar.activation(gt[:, mc, :], psg[:], AF.Sigmoid)

                    cat = mcat.tile([128, 3, 512], F32R, tag="cat", name="cat")
                    sq = msq.tile([128, 3, 512], F32R, tag="sq", name="sq")

                    def evict(ps, k, rows):
                        nc.any.tensor_copy(cat[rows, k, :], ps[:])
                        # square from the SBUF copy (not psum) so the op can
                        # land on either DVE or ACT, whichever is less busy
                        nc.any.tensor_tensor(
                            sq[rows, k, :], cat[rows, k, :], cat[rows, k, :],
                            OP.mult,
                        )

                    ps_cha = pwave.tile([64, 512], F32, tag="cw", name="ps_cha")
                    nc.tensor.matmul(
                        ps_cha[:], att0[:, 0:64], x2T0[:, ng : ng + 512],
                        start=True, stop=False,
                    )
                    nc.tensor.matmul(
                        ps_cha[:], att1[:, 0:64], x2T1p[0:64, nb : nb + 512],
                        start=False, stop=True,
                    )
                    ps_chb = pwave.tile([128, 512], F32, tag="cw", name="ps_chb")
                    nc.tensor.matmul(
                        ps_chb[:], att0[:, 64:192], x2T0[:, ng : ng + 512],
                        start=True, stop=False,
                    )
                    nc.tensor.matmul(
                        ps_chb[:], att1[:, 64:192], x2T1p[0:64, nb : nb + 512],
                        start=False, stop=True,
                    )
                    evict(ps_cha, 1, slice(64, 128))
                    evict(ps_chb, 2, slice(0, 128))

                    ps_sp0 = pwave.tile([128, 512], F32, tag="cw", name="ps_sp0")
                    for mc in range(8):
                        nc.tensor.matmul(
                            ps_sp0[:], x2cT[:, mc, 0:128], gt[:, mc, :],
                            start=(mc == 0), stop=(mc == 7),
                        )
                    ps_sp1 = pwave.tile([64, 512], F32, tag="cw", name="ps_sp1")
                    for mc in range(8):
                        nc.tensor.matmul(
                            ps_sp1[:], x2cT[:, mc, 128:192], gt[:, mc, :],
                            start=(mc == 0), stop=(mc == 7),
                        )
                    evict(ps_sp0, 0, slice(0, 128))
                    evict(ps_sp1, 1, slice(0, 64))

                    ps_s1 = pproj.tile([1, 512], F32, tag="pp", name="ps_s1")
                    for k in range(3):
                        nc.tensor.matmul(
                            ps_s1[:], invC_col[:], cat[:, k, :],
                            start=(k == 0), stop=(k == 2),
                        )
                    ps_s2 = pproj.tile([1, 512], F32, tag="pp", name="ps_s2")
                    for k in range(3):
                        nc.tensor.matmul(
                            ps_s2[:], invC_col[:], sq[:, k, :],
                            start=(k == 0), stop=(k == 2),
                        )
                    # mu_r / var rows (everything except the Sqrt)
                    mu_r = mrows.tile([1, 512], F32R, tag="rowB", name="mu_r")
                    nc.vector.tensor_copy(mu_r[:], ps_s1[:])
                    musq = mrows.tile([1, 512], F32, tag="rowC", name="musq")
                    nc.scalar.activation(musq[:], ps_s1[:], AF.Square)
                    nc.vector.tensor_tensor(musq[:], ps_s2[:], musq[:], OP.subtract)
                    return cat, mu_r, musq

                def emit_sqrt(musq):
                    # Sqrt ops for the pair are adjacent -> one ACT table
                    # round-trip per pair instead of per block
                    nc.scalar.activation(musq[:], musq[:], AF.Sqrt, bias=eps_sb[:])

                def emit_back(blk, cat, mu_r, musq):
                    nb = 512 * blk
                    rstd_r = mrows.tile([1, 512], F32R, tag="rowD", name="rstd_r")
                    with nc.allow_low_precision(reason="fp32r rstd, 4-byte"):
                        nc.vector.reciprocal(rstd_r[:], musq[:])

                    ofin = mout.tile([128, 3, 512], F32, tag="ofin", name="ofin")
                    psos = []
                    for oc in range(2):
                        pso = pproj.tile([128, 512], F32, tag="pp", name="pso")
                        psos.append(pso)
                        for k in range(3):
                            nc.tensor.matmul(
                                pso[:],
                                pwTs[k][:, 128 * oc : 128 * (oc + 1)],
                                cat[:, k, :],
                                start=(k == 0), stop=False,
                            )
                    ps_bc = pmisc.tile([128, 512], F32, tag="pm", name="ps_bc")
                    nc.tensor.matmul(
                        ps_bc[:], onesr_r[:], rstd_r[:], start=True, stop=True
                    )
                    rstd_bc = mout.tile([128, 512], F32, tag="rstd_bc", name="rstd_bc")
                    nc.any.tensor_copy(rstd_bc[:], ps_bc[:])

                    def finish_oc(oc, pso):
                        nc.tensor.matmul(
                            pso[:],
                            pwsumneg_row[:, 128 * oc : 128 * (oc + 1)],
                            mu_r[:],
                            start=False, stop=True,
                        )
                        osb = mout.tile([128, 512], F32, tag="osb", name="osb")
                        nc.vector.tensor_tensor(osb[:], pso[:], rstd_bc[:], OP.mult)
                        nc.any.tensor_scalar_add(
                            ofin[:, oc, :], osb[:], bias2_sb[:, oc : oc + 1]
                        )

                    finish_oc(0, psos[0])
                    finish_oc(1, psos[1])
                    pso2 = pproj.tile([128, 512], F32, tag="pp", name="pso2")
                    for k in range(3):
                        nc.tensor.matmul(
                            pso2[:],
                            pwTs[k][:, 256 : 384],
                            cat[:, k, :],
                            start=(k == 0), stop=False,
                        )
                    finish_oc(2, pso2)
                    nc.sync.dma_start(
                        out_part.ap().rearrange(
                            "(oc p) n -> p oc n", p=128
                        )[:, :, nb : nb + 512],
                        ofin[:],
                    )

                for pair in range(NBLOCKS // 2):
                    A, Bb = 2 * pair, 2 * pair + 1
                    next_x1t[Bb] = make_x1t(Bb)
                    hA = emit_front(A)
                    if A + 2 < NBLOCKS:
                        next_x1t[A + 2] = make_x1t(A + 2)
                    hB = emit_front(Bb)
                    emit_sqrt(hA[2])
                    emit_sqrt(hB[2])
                    emit_back(A, *hA)
                    emit_back(Bb, *hB)

    nc.finalize()
    return nc


_NC_CACHE: dict = {}


def _get_nc():
    if "nc" not in _NC_CACHE:
        _NC_CACHE["nc"] = build_nc()
    return _NC_CACHE["nc"]


def _prep_in_map(x_b, conv_w, conv_b, ln_w, ln_b, proj_w, proj_b):
    convw_t = np.ascontiguousarray(conv_w.transpose(1, 2, 3, 0)).reshape(D, 16, D)
    pwT = np.ascontiguousarray(proj_w.T)
    return {
        "x_full": np.ascontiguousarray(x_b, dtype=np.float32),
        "convw": convw_t.astype(np.float32),
        "convb": np.asarray(conv_b, dtype=np.float32),
        "lnw": np.asarray(ln_w, dtype=np.float32),
        "lnb": np.asarray(ln_b, dtype=np.float32),
        "pwT": pwT.astype(np.float32),
        "pb": np.asarray(proj_b, dtype=np.float32),
    }


def kernel(x, conv_w, conv_b, ln_w, ln_b, proj_w, proj_b, H=128, W=128):
    """Full-input entry point: shards over 8 cores (4 samples x 2 N-halves),
    runs the Bass kernel, gathers the full [B, C, N] output."""
    from concourse.bass_utils import run_bass_kernel_spmd

    x = np.asarray(x)
    assert x.shape == (B, N, C), x.shape

    # Core 2b + h handles (sample b, N-half h). Half-1 cores get the two
    # N-halves of x swapped so every core computes "rows 0:8192".
    nc = _get_nc()
    in_maps = []
    for b in range(B):
        for half in (0, 1):
            xb = x[b] if half == 0 else np.concatenate(
                [x[b, NH:], x[b, :NH]], axis=0
            )
            in_maps.append(
                _prep_in_map(xb, np.asarray(conv_w), np.asarray(conv_b),
                             np.asarray(ln_w), np.asarray(ln_b),
                             np.asarray(proj_w), np.asarray(proj_b))
            )
    res = run_bass_kernel_spmd(nc, in_maps, core_ids=list(range(8)))

    out = np.empty((B, C, N), dtype=np.float32)
    for b in range(B):
        for half in (0, 1):
            out[b][:, half * NH : (half + 1) * NH] = \
                res.results[2 * b + half]["out_part"]
    return out
